# revision 30
# baseline (speedup 1.0000x reference)
"""Trainium2 Bass kernel: conv2d(3x3, VALID) + bias -> channel-min -> tanh(tanh).

Full inputs in, full output out. Data-parallel over batch across 8 NeuronCores.

Per-core scheme (weight-stationary conv as matmul + log-sum-exp channel-min):
  - min over channels commutes with the monotone tanh(tanh(.)), and
    min_c y_c = -(1/p) ln sum_c exp(-p y_c) to within ln(64)/p. With p=12
    the end-to-end error lands ~1e-3 relative (gate is 2e-2): the conv
    output's min is ~-2.2 sigma where d/dx tanh(tanh(x)) ~ 0.02, so LSE
    error is crushed 50x. This replaces the channel-min transpose +
    min-tree (the baseline's DMA-descriptor bottleneck: ~135k xbar
    descriptors) with one Exp evacuation and one ones-vector matmul --
    the partition reduction PE can do natively.
  - Conv: output rows processed in (delta, t) pairs, h' = 2t + delta.
    Matmul M packs (delta, oc): M = 128. Contraction K packs (khe, ic),
    khe = delta + kh in [0,4): K = 64. 3 PSUM-accumulated matmuls per
    group (kw as free-dim offset into row-shifted image copies built on
    host). Two images run concurrently on disjoint PE row halves via
    tile_position row tiling.
  - Groups of 4 row-pairs (N = 512) are processed in quads (2 groups x
    2 halves -> one [128, 2048] f32 PSUM tile = 4 banks, double-buffered
    = all 8 banks).
  - One ScalarE Exp per quad evacuates PSUM -> SBUF bf16:
    e = exp(-12*(psum + b)) via the activation's free scale/bias affine.
  - Channel sum: 4 matmuls per quad with a [128, 2] ones-selector lhsT
    (sel[(d,oc), m] = d==m), N = 512, col-tiled to partition slots
    {0,32,64,96} of bank 0 of the (by then drained) conv PSUM tile.
  - VectorE evacuates the sparse [98, 512] sums into a per-pair strip,
    fused with a bit-pattern ln (u = (bits(s)*2^-23 - B)*ln2); bulk DMAs
    bounce the strip through DRAM to repack partitions densely.
  - Finals per 2-pair chunk: two table Tanh ops on ScalarE (the kernel
    uses only exp+tanh = one ACT table set, no ACT_TABLE_LOAD switches).
  - Pipeline: conv of quad q overlaps exp/schraudolph of q and the
    lagged sums/sv of q-1 (separate 1-bank PSUM tile so the sv WAR
    never blocks conv); chunk finals deferred into the next pair.
  - Output rows (d, q, gl, tt)-major; host reorders to h' = 2t + delta.
"""

import os
import sys

for _p in ("/opt/trn_rl_repo", "/root/.axon_site/_ro/trn_rl_repo"):
    if os.path.isdir(_p) and _p not in sys.path:
        sys.path.insert(0, _p)

import numpy as np
import ml_dtypes

import concourse.bass as bass
import concourse.bacc as bacc
import concourse.tile as tile
from concourse import mybir
from concourse.bass_utils import run_bass_kernel_spmd

N_CORES = 8
B, IC, H, W = 128, 16, 128, 128
OC, KSZ = 64, 3
HO, WO = H - KSZ + 1, W - KSZ + 1  # 126, 126
B_LOC = B // N_CORES  # 16
PAIRS = B_LOC // 2  # 8
T = HO // 2  # 63 row-pairs per image (h' = 2t + delta)
FLAT = H * W  # 16384
P_LSE = 12.0

BF16 = mybir.dt.bfloat16
F32 = mybir.dt.float32
F8 = mybir.dt.float8e4
# fp8 runs the PE at bf16 speed but halves SBUF + HBM traffic. Weights are
# scaled x16 so the bulk of them sit above the e4m3 denormal cliff (2^-6);
# the 1/16 is folded into the exp scales downstream.
W_SCALE = 16.0

# t-groups of 4 row-pairs -> conv matmul N = 512 always. The last group's
# t=63 is a dummy: it reads the (valid, zero-padded) image tail so every
# PSUM byte is freshly written each quad; its results are computed but
# never packed (pack uses the real cnt = min(4, T - t0)).
GROUPS = [(t0, min(4, T - t0)) for t0 in range(0, T, 4)]  # 16 groups, last cnt=3
QUADS = [(GROUPS[2 * i], GROUPS[2 * i + 1]) for i in range(8)]
NCHUNK = PAIRS // 2  # finals batched per 2 pairs
# exp evac split: ScalarE ACT does psm[0:ESPLIT]; VectorE Schraudolph does
# the psz slot. V also carries the strip evac + psz WARs sit on V, so
# shifting exp columns onto V stalls the conv pipeline -- keep S-heavy.
ESPLIT = 1536


def _build_program():
    nc = bacc.Bacc(None)
    xr_hbm = nc.declare_dram_parameter(
        "xrep", [PAIRS, 128, FLAT], F8, isOutput=False
    )
    w_hbm = nc.declare_dram_parameter("wts", [128, 3 * 128], F8, isOutput=False)
    b_hbm = nc.declare_dram_parameter("bias", [128, 1], F32, isOutput=False)
    b2_hbm = nc.declare_dram_parameter("bias2", [128, 1], F32, isOutput=False)
    sel_hbm = nc.declare_dram_parameter("sel", [128, 2], BF16, isOutput=False)
    # y rows are (d, q, gl, tt)-major: h' = 2*(8q + 4gl + tt) + d;
    # rows 63 and 127 are the dummy t=63 (host drops them)
    y_hbm = nc.declare_dram_parameter("y", [NCHUNK, 128, 512], F32, isOutput=True)

    with tile.TileContext(nc) as tc:
        with (
            tc.tile_pool(name="const", bufs=1) as const,
            tc.tile_pool(name="xrp", bufs=2) as xrp,
            tc.tile_pool(name="psm", bufs=2, space="PSUM") as psmain,
            tc.tile_pool(name="psz", bufs=2, space="PSUM") as psb0,
            tc.tile_pool(name="ep", bufs=3) as ep,
            tc.tile_pool(name="svp", bufs=3) as svp,
            tc.tile_pool(name="finp", bufs=2) as finp,
            tc.tile_pool(name="stgp", bufs=2, space="DRAM") as stgp,
            tc.tile_pool(name="tmpp", bufs=7) as tmpp,
        ):
            w_sb = const.tile([128, 3 * 128], F8)
            b_sb = const.tile([128, 1], F32)
            b2_sb = const.tile([128, 1], F32)
            sel_sb = const.tile([128, 2], BF16)
            nc.sync.dma_start(w_sb[:], w_hbm[:])
            nc.sync.dma_start(b_sb[:], b_hbm[:])
            nc.sync.dma_start(b2_sb[:], b2_hbm[:])
            nc.sync.dma_start(sel_sb[:], sel_hbm[:])

            xr_tiles = {}

            def load_pair_slice(p, q, eng=None):
                # quad-granular prefetch slices on the gpsimd HWDGE ring:
                # each dma_start costs ~600-760ns of issuing-engine FIFO
                # time, and gpsimd is otherwise idle while scalar's ACT
                # stream is a co-bottleneck. Slices are issued a full pair
                # (~8 quads) ahead, so gpsimd's slower triggers don't gate
                # anything. One 256KB slice per quad bounds the backlog.
                if q == 0:
                    xr_tiles[p] = xrp.tile([128, FLAT], F8, name="xr", tag="xr")
                xr_t = xr_tiles[p]
                sl = FLAT // 8
                (eng or nc.gpsimd).dma_start(
                    xr_t[:, q * sl : (q + 1) * sl], xr_hbm[p, :, q * sl : (q + 1) * sl]
                )

            # pair 0 is on the critical path at startup: fan its slices out
            # over two rings (consts ride sync) so the PE isn't data-starved.
            for q in range(8):
                load_pair_slice(0, q, (nc.scalar, nc.gpsimd)[q % 2])

            # pending_red = quad-lagged reduction work; pending_work =
            # (due_pair, due_qi, fn) items for the chunk-finals pipeline,
            # spread out so the fin-load lands quads before its tanhs ever
            # enter the ACT FIFO (head-of-line blocking the exps was worth
            # ~4.7us per chunk in the trace).
            pending_red = None
            pending_work = []
            fin = None
            stgv = None

            # conv slot map: (half, gl) -> (tile, offset, e-offset).
            # Three slots live in the 3-bank "main" tile; the 4th (h0,gl0)
            # lives in its own 1-bank tile that also receives the channel
            # sums + sv evac -- so the sv WAR chain never blocks the main
            # conv tile of the next quad (Tile WARs are tile-granular).
            def slot_info(half, gl, psm, psz):
                idx = {(0, 1): 0, (1, 0): 1, (1, 1): 2}.get((half, gl))
                if idx is None:
                    return psz, 0, 1536
                return psm, idx * 512, idx * 512

            def emit_conv(xrv, psm, psz, quad):
                # 12 matmuls, row-half interleaved for PE row-tiling
                # overlap; the psz slot is in the FIRST wave so its
                # Schraudolph evac can start a wave early.
                for wave in range(2):
                    for kw in range(3):
                        for half in range(2):
                            gl = half if wave == 0 else (1 - half)
                            rl, rh = 64 * half, 64 * half + 64
                            t0 = quad[gl][0]
                            pst, off, _ = slot_info(half, gl, psm, psz)
                            nc.tensor.matmul(
                                pst[:, off : off + 512],
                                w_sb[rl:rh, kw * 128 : (kw + 1) * 128],
                                xrv[rl:rh, t0 : t0 + 4, kw : kw + 128],
                                start=(kw == 0),
                                stop=(kw == 2),
                                tile_position=(64 * half, 0),
                                skip_group_check=True,
                            )

            def emit_reduce(psz, e, quad, qi, strip):
                # channel sums of quad q-1, written into the CURRENT quad's
                # psz bank (after its Schraudolph read): the sv WAR on the
                # psz pool then spans two generations and never stalls conv.
                # col-tiled [2, 512] matmuls into the psz bank
                for gl, (t0, cnt) in enumerate(quad):
                    for half in range(2):
                        _, _, eoff = slot_info(half, gl, None, None)
                        j = 32 * (2 * gl + half)
                        nc.tensor.matmul(
                            psz[j : j + 2, 0:512],
                            sel_sb[:, 0:2],
                            e[:, eoff : eoff + 512],
                            start=True,
                            stop=True,
                            tile_position=(0, j),
                            skip_group_check=True,
                        )
                # sparse evac of the sums on VectorE into the per-pair
                # strip, fused with the bit-pattern ln:
                # u = ln(s) ~= (bits(s)*2^-23 - B)*ln2
                # (copies some conv garbage on unused partitions; unread)
                ln2 = float(np.log(2.0))
                nc.vector.tensor_scalar(
                    strip[0:98, qi * 512 : (qi + 1) * 512],
                    psz[0:98, 0:512].bitcast(mybir.dt.int32),
                    ln2 / (1 << 23),
                    -(127.0 - 0.0430) * ln2,
                    mybir.AluOpType.mult,
                    mybir.AluOpType.add,
                )

            def emit_pair_stores(strip, pl, my_stgv, q_lo, q_hi):
                # repack bounce 1/2: 8 bulk scatters per pair-half (not 64
                # tiny ones -- each small DMA costs ~620ns of HWDGE ring
                # time and the backlog starved the chunk fin-loads). The
                # stage rows are (d, q, gl, tt)-major (row 8q+4gl+tt+64d;
                # the dummy t=63 lands in rows 63/127), so each
                # (gl, half, d) is one clean 3-dim transfer:
                # strip[32*(2gl+h)+d, q*512 + f] -> stage[d,q,gl][f-block]
                # Emitted in two q-halves so the ring load spreads out and
                # the tail drain only waits for the second half.
                # the d=0/d=1 rows ride in ONE transfer (partition dim 2 on
                # the SBUF side): each dma_start costs ~600ns on the issuing
                # engine's FIFO, so 4 triggers per half beat 8. Stage rows
                # are (d, gl, q, tt)-major so (q, tt) is contiguous and the
                # AP stays 3-dim after balancing.
                for gl in range(2):
                    for half in range(2):
                        j = 32 * (2 * gl + half)
                        wo = (2 * pl + half) * 128
                        sview = strip[j : j + 2, :].rearrange(
                            "o (q tt w) -> o q tt w", q=8, tt=4
                        )[:, q_lo:q_hi]
                        dview = my_stgv[:, gl, q_lo:q_hi, :, wo : wo + 128]
                        nc.sync.dma_start(dview, sview)

            def sched_chunk_finals(my_fin, my_stg, my_pair):
                # finals pipeline for the 2-pair chunk on [126, 512]: fin
                # holds u = ln(s) (bit-pattern ln fused into the sv evac);
                # min' = -u/12 ; out = tanh(tanh(min')). Both tanhs come
                # from the ACT table -- the whole kernel stays in the
                # exp_and_others set (exp + tanh): no ACT_TABLE_LOAD
                # switches, no slow VectorE reciprocals. The three stages
                # are spread across the NEXT pair's quads so the tanhs
                # only enter the strict-FIFO ACT queue once their input is
                # already resident in SBUF.
                r = tmpp.tile([128, 512], F32, name="t", tag="t")
                o = tmpp.tile([128, 512], F32, name="t", tag="t")

                def load():
                    # repack bounce 2/2: one dense load per chunk
                    nc.sync.dma_start(my_fin[0:128, :], my_stg[:, :])

                def tanh1():
                    nc.scalar.activation(
                        r[0:128, :], my_fin[0:128, :],
                        mybir.ActivationFunctionType.Tanh,
                        scale=-1.0 / P_LSE,
                    )

                def tanh2():
                    nc.scalar.activation(
                        o[0:128, :], r[0:128, :],
                        mybir.ActivationFunctionType.Tanh,
                    )
                    nc.sync.dma_start(y_hbm[my_pair // 2], o[0:128, :])

                pending_work.append((my_pair + 1, 1, load))
                pending_work.append((my_pair + 1, 4, tanh1))
                pending_work.append((my_pair + 1, 5, tanh2))

            def sched_last_chunk_half(my_fin, my_stg, my_pair, hw, due):
                # per-pair-half finals for the LAST chunk: the pair-6 half
                # runs during pair 7's quads; only pair 7's quarter rides
                # the serial drain. hw = half-window (0 or 1) over columns.
                c0, c1 = hw * 256, hw * 256 + 256
                r = tmpp.tile([128, 256], F32, name="t", tag="t")
                o = tmpp.tile([128, 256], F32, name="t", tag="t")

                def load():
                    nc.sync.dma_start(my_fin[0:128, c0:c1], my_stg[:, c0:c1])

                def tanh1():
                    nc.scalar.activation(
                        r[0:128, :], my_fin[0:128, c0:c1],
                        mybir.ActivationFunctionType.Tanh,
                        scale=-1.0 / P_LSE,
                    )

                def tanh2():
                    nc.scalar.activation(
                        o[0:128, :], r[0:128, :],
                        mybir.ActivationFunctionType.Tanh,
                    )
                    nc.sync.dma_start(y_hbm[my_pair // 2][:, c0:c1], o[0:128, :])

                pending_work.append((due[0], due[1], load))
                pending_work.append((due[0], due[1] + 2, tanh1))
                pending_work.append((due[0], due[1] + 3, tanh2))

            for pair in range(PAIRS):
                xr = xr_tiles.pop(pair)
                # free dim as 64 double-rows of 256 (row r=2t at offset t*256)
                xrv = xr.rearrange("p (r q) -> p r q", q=2 * W)
                pl = pair % 2  # slot within the 2-pair finals chunk
                if pl == 0:
                    fin = finp.tile([128, 512], F32, name="fin", tag="fin")
                    stg = stgp.tile([128, 512], F32, name="stg", tag="stg")
                    stgv = stg.rearrange(
                        "(d gl q tt) w -> d gl q tt w", d=2, gl=2, q=8
                    )
                strip = svp.tile([128, 8 * 512], F32, name="strip", tag="strip")
                if pair == PAIRS - 1:
                    # pair-6 half of the last chunk: its stores land at
                    # (pair 7, qi 0); run load/tanhs during pair 7's quads
                    sched_last_chunk_half(fin, stg, pair, 0, (pair, 1))

                for qi, quad in enumerate(QUADS):
                    psm = psmain.tile([128, 1536], F32, name="psm")
                    psz = psb0.tile([128, 512], F32, name="psz")
                    emit_conv(xrv, psm, psz, quad)
                    if pair + 1 < PAIRS:
                        load_pair_slice(pair + 1, qi)
                    # e = exp(-12*(conv + b)), split across two engines:
                    # ScalarE Exp on the 3-slot main tile, VectorE
                    # Schraudolph bf16-exp on the psz slot (affine in f32,
                    # stored as int16 whose bits form bf16(e^z)).
                    e = ep.tile([128, 2048], BF16, name="e", tag="e")
                    nc.scalar.activation(
                        e[:, 0:1536],
                        psm[:, 0:1536],
                        mybir.ActivationFunctionType.Exp,
                        bias=b_sb[:, 0:1],
                        scale=-P_LSE / W_SCALE,
                    )
                    nc.vector.tensor_scalar(
                        e[:, 1536:2048].bitcast(mybir.dt.int16),
                        psz[:, 0:512],
                        -P_LSE / W_SCALE * 128.0 / float(np.log(2.0)),
                        b2_sb[:, 0:1],
                        mybir.AluOpType.mult,
                        mybir.AluOpType.add,
                    )
                    # quad-lagged reduction of the PREVIOUS quad, targeting
                    # THIS quad's psz tile (its schr read is done by then):
                    # exp(q-1) finished during this conv, so the sums never
                    # stall, and the sv WAR spans two psz generations.
                    if pending_red is not None:
                        p_e, p_quad, p_qi, p_strip, p_pl, p_stgv = pending_red
                        emit_reduce(psz, p_e, p_quad, p_qi, p_strip)
                        if p_qi == 3:
                            emit_pair_stores(p_strip, p_pl, p_stgv, 0, 4)
                        elif p_qi == 6 and pair == PAIRS - 1:
                            # flush the last pair's [4,7) strip quads early
                            # so the drain only waits on quad 7's quarter
                            emit_pair_stores(p_strip, p_pl, p_stgv, 4, 7)
                        elif p_qi == 7:
                            emit_pair_stores(p_strip, p_pl, p_stgv, 4, 8)
                    pending_red = (e, quad, qi, strip, pl, stgv)
                    # run any due finals-pipeline stages (load / tanh1 /
                    # tanh2+store), emitted at the tail of the quad so the
                    # quad's own exp sits ahead of them in the ACT FIFO.
                    for item in list(pending_work):
                        dp, dq, fn = item
                        if pair > dp or (pair == dp and qi >= dq):
                            fn()
                            pending_work.remove(item)

                if pl == 1:
                    if pair == PAIRS - 1:
                        sched_last_chunk_half(fin, stg, pair, 1, (PAIRS, 0))
                    else:
                        sched_chunk_finals(fin, stg, pair)


            # drain: last quad's reduction (fresh psz bank, zeroed so the
            # sv evac's garbage partitions are initialized) + last pair's
            # stores + the remaining finals stages in order
            if pending_red is not None:
                p_e, p_quad, p_qi, p_strip, p_pl, p_stgv = pending_red
                psz_last = psb0.tile([128, 512], F32, name="psz")
                nc.vector.memset(psz_last[:, :], 0.0)
                emit_reduce(psz_last, p_e, p_quad, p_qi, p_strip)
                # [4,7) was flushed during quad 7; only quad 7's quarter left
                emit_pair_stores(p_strip, p_pl, p_stgv, 7, 8)
            for _, _, fn in pending_work:
                fn()
            pending_work.clear()
    nc.finalize()
    return nc


_NC_CACHE = None


def _get_program():
    global _NC_CACHE
    if _NC_CACHE is None:
        _NC_CACHE = _build_program()
    return _NC_CACHE


def _host_prep(x, conv_weight, conv_bias):
    # x: [B, IC, H, W] f32
    # xrep[b, khe, ic, r, :] = x[b, ic, r+khe, :]  (zero past the end)
    xb = x.astype(ml_dtypes.float8_e4m3)
    xrep = np.zeros((B, 4, IC, H, W), dtype=ml_dtypes.float8_e4m3)
    for khe in range(4):
        xrep[:, khe, :, : H - khe, :] = xb[:, :, khe:, :]
    xrep = xrep.reshape(B, 4 * IC, FLAT)

    # weights: Wl[p=(khe*16+ic), kw, m=(delta*64+oc)] = w[oc, ic, khe-delta, kw]
    wl = np.zeros((64, 3, 128), dtype=np.float32)
    for khe in range(4):
        for dlt in range(2):
            kh = khe - dlt
            if 0 <= kh < KSZ:
                wl[khe * 16 : khe * 16 + 16, :, dlt * 64 : dlt * 64 + 64] = (
                    conv_weight[:, :, kh, :].transpose(1, 2, 0)
                )
    wts = np.concatenate([wl, wl], axis=0).reshape(128, 3 * 128) * W_SCALE
    wts = wts.astype(ml_dtypes.float8_e4m3)

    biasarr = np.tile(conv_bias.astype(np.float32), 2).reshape(128, 1)
    biasarr = biasarr * (-P_LSE)  # ACT bias applied after scale: exp(s*x + b)

    # Schraudolph bias for the VectorE exp path: bits(bf16 e^z) ~=
    # z*128/ln2 + 127*128 + c, z = -12*(psum + b). c centers the
    # mantissa-linear error and absorbs the f32->int16 truncation.
    bias2 = biasarr * (128.0 / np.log(2.0)) + (127.0 * 128.0 + 6.0)
    bias2 = bias2.astype(np.float32)

    # ones selector: sel[(d, oc), m] = (d == m)
    sel = np.zeros((128, 2), dtype=np.float32)
    sel[0:64, 0] = 1.0
    sel[64:128, 1] = 1.0
    sel = sel.astype(ml_dtypes.bfloat16)
    return xrep, wts, biasarr, bias2, sel


def _build_in_maps(x, conv_weight, conv_bias):
    xrep, wts, biasarr, bias2, sel = _host_prep(x, conv_weight, conv_bias)
    in_maps = []
    for c in range(N_CORES):
        xc = xrep[c * B_LOC : (c + 1) * B_LOC]  # [B_LOC, 64, FLAT]
        xc = np.ascontiguousarray(xc).reshape(PAIRS, 128, FLAT)
        in_maps.append(
            {"xrep": xc, "wts": wts, "bias": biasarr, "bias2": bias2, "sel": sel}
        )
    return in_maps


def kernel(x, conv_weight, conv_bias):
    x = np.asarray(x, dtype=np.float32)
    conv_weight = np.asarray(conv_weight, dtype=np.float32)
    conv_bias = np.asarray(conv_bias, dtype=np.float32)

    in_maps = _build_in_maps(x, conv_weight, conv_bias)
    nc = _get_program()
    res = run_bass_kernel_spmd(nc, in_maps, list(range(N_CORES)))
    # y: [NCHUNK, 128, 512]; rows (d 2, gl 2, q 8, tt 4) with
    # h' = 2*(8q + 4gl + tt) + d (t=63 dummy rows dropped);
    # cols (pair_loc 2, half 2, w 128); image b = chunk*4 + pl*2 + half
    ys = []
    for c in range(N_CORES):
        yc = res.results[c]["y"].reshape(NCHUNK, 2, 2, 8, 4, 2, 2, 128)
        # [chunk, d, gl, q, tt, pl, half, w] -> [chunk, pl, half, q, gl, tt, d, w]
        yc = yc.transpose(0, 5, 6, 3, 2, 4, 1, 7)
        ys.append(yc.reshape(B_LOC, 64, 2, 128))
    y = np.concatenate(ys, axis=0)  # [B, t, d, w]
    y = y.reshape(B, 128, 128)[:, :HO, :WO]  # (t,d) = h', drop dummies
    return np.ascontiguousarray(y).reshape(B, 1, HO, WO).astype(np.float32)



# revision 42
# speedup vs baseline: 1.0177x; 1.0177x over previous
"""Trainium2 Bass kernel: conv2d(3x3, VALID) + bias -> channel-min -> tanh(tanh).

Full inputs in, full output out. Data-parallel over batch across 8 NeuronCores.

Per-core scheme (weight-stationary conv as matmul + log-sum-exp channel-min):
  - min over channels commutes with the monotone tanh(tanh(.)), and
    min_c y_c = -(1/p) ln sum_c exp(-p y_c) to within ln(64)/p. With p=12
    the end-to-end error lands ~1e-3 relative (gate is 2e-2): the conv
    output's min is ~-2.2 sigma where d/dx tanh(tanh(x)) ~ 0.02, so LSE
    error is crushed 50x. This replaces the channel-min transpose +
    min-tree (the baseline's DMA-descriptor bottleneck: ~135k xbar
    descriptors) with one Exp evacuation and one ones-vector matmul --
    the partition reduction PE can do natively.
  - Conv: output rows processed in (delta, t) pairs, h' = 2t + delta.
    Matmul M packs (delta, oc): M = 128. Contraction K packs (khe, ic),
    khe = delta + kh in [0,4): K = 64. 3 PSUM-accumulated matmuls per
    group (kw as free-dim offset into row-shifted image copies built on
    host). Two images run concurrently on disjoint PE row halves via
    tile_position row tiling.
  - Groups of 4 row-pairs (N = 512) are processed in quads (2 groups x
    2 halves -> one [128, 2048] f32 PSUM tile = 4 banks, double-buffered
    = all 8 banks).
  - One ScalarE Exp per quad evacuates PSUM -> SBUF bf16:
    e = exp(-12*(psum + b)) via the activation's free scale/bias affine.
  - Channel sum: 4 matmuls per quad with a [128, 2] ones-selector lhsT
    (sel[(d,oc), m] = d==m), N = 512, col-tiled to partition slots
    {0,32,64,96} of bank 0 of the (by then drained) conv PSUM tile.
  - VectorE evacuates the sparse [98, 512] sums into a per-pair strip,
    fused with a bit-pattern ln (u = (bits(s)*2^-23 - B)*ln2); bulk DMAs
    bounce the strip through DRAM to repack partitions densely.
  - Finals per 2-pair chunk: two table Tanh ops on ScalarE (the kernel
    uses only exp+tanh = one ACT table set, no ACT_TABLE_LOAD switches).
  - Pipeline: conv of quad q overlaps exp/schraudolph of q and the
    lagged sums/sv of q-1 (separate 1-bank PSUM tile so the sv WAR
    never blocks conv); chunk finals deferred into the next pair.
  - Output rows (d, q, gl, tt)-major; host reorders to h' = 2t + delta.
"""

import os
import sys

for _p in ("/opt/trn_rl_repo", "/root/.axon_site/_ro/trn_rl_repo"):
    if os.path.isdir(_p) and _p not in sys.path:
        sys.path.insert(0, _p)

import numpy as np
import ml_dtypes

import concourse.bass as bass
import concourse.bacc as bacc
import concourse.tile as tile
from concourse import mybir
from concourse.bass_utils import run_bass_kernel_spmd

N_CORES = 8
B, IC, H, W = 128, 16, 128, 128
OC, KSZ = 64, 3
HO, WO = H - KSZ + 1, W - KSZ + 1  # 126, 126
B_LOC = B // N_CORES  # 16
PAIRS = B_LOC // 2  # 8
T = HO // 2  # 63 row-pairs per image (h' = 2t + delta)
FLAT = H * W  # 16384
P_LSE = 12.0

BF16 = mybir.dt.bfloat16
F32 = mybir.dt.float32
F8 = mybir.dt.float8e4
# fp8 runs the PE at bf16 speed but halves SBUF + HBM traffic. Weights are
# scaled x16 so the bulk of them sit above the e4m3 denormal cliff (2^-6);
# the 1/16 is folded into the exp scales downstream.
W_SCALE = 16.0

# t-groups of 4 row-pairs -> conv matmul N = 512 always. The last group's
# t=63 is a dummy: it reads the (valid, zero-padded) image tail so every
# PSUM byte is freshly written each quad; its results are computed but
# never packed (pack uses the real cnt = min(4, T - t0)).
GROUPS = [(t0, min(4, T - t0)) for t0 in range(0, T, 4)]  # 16 groups, last cnt=3
QUADS = [(GROUPS[2 * i], GROUPS[2 * i + 1]) for i in range(8)]
NCHUNK = PAIRS // 2  # finals batched per 2 pairs
# exp evac split: ScalarE ACT does psm[0:ESPLIT]; VectorE Schraudolph does
# the psz slot. V also carries the strip evac + psz WARs sit on V, so
# shifting exp columns onto V stalls the conv pipeline -- keep S-heavy.
ESPLIT = 1536


def _build_program():
    nc = bacc.Bacc(None)
    xr_hbm = nc.declare_dram_parameter(
        "xrep", [PAIRS, 128, FLAT], F8, isOutput=False
    )
    w_hbm = nc.declare_dram_parameter("wts", [128, 3 * 128], F8, isOutput=False)
    b_hbm = nc.declare_dram_parameter("bias", [128, 1], F32, isOutput=False)
    b2_hbm = nc.declare_dram_parameter("bias2", [128, 1], F32, isOutput=False)
    sel_hbm = nc.declare_dram_parameter("sel", [128, 2], BF16, isOutput=False)
    # y rows are (d, q, gl, tt)-major: h' = 2*(8q + 4gl + tt) + d;
    # rows 63 and 127 are the dummy t=63 (host drops them)
    y_hbm = nc.declare_dram_parameter("y", [NCHUNK, 128, 512], F32, isOutput=True)

    with tile.TileContext(nc) as tc:
        with (
            tc.tile_pool(name="const", bufs=1) as const,
            tc.tile_pool(name="xrp", bufs=2) as xrp,
            tc.tile_pool(name="psm", bufs=2, space="PSUM") as psmain,
            tc.tile_pool(name="psz", bufs=2, space="PSUM") as psb0,
            tc.tile_pool(name="ep", bufs=3) as ep,
            tc.tile_pool(name="svp", bufs=3) as svp,
            tc.tile_pool(name="finp", bufs=2) as finp,
            tc.tile_pool(name="stgp", bufs=2, space="DRAM") as stgp,
            tc.tile_pool(name="tmpp", bufs=7) as tmpp,
        ):
            w_sb = const.tile([128, 3 * 128], F8)
            b_sb = const.tile([128, 1], F32)
            b2_sb = const.tile([128, 1], F32)
            sel_sb = const.tile([128, 2], BF16)
            nc.sync.dma_start(w_sb[:], w_hbm[:])
            nc.sync.dma_start(b_sb[:], b_hbm[:])
            nc.sync.dma_start(b2_sb[:], b2_hbm[:])
            nc.sync.dma_start(sel_sb[:], sel_hbm[:])

            xr_tiles = {}

            def load_pair_slice(p, q, eng=None):
                # quad-granular prefetch slices on the gpsimd HWDGE ring:
                # each dma_start costs ~600-760ns of issuing-engine FIFO
                # time, and gpsimd is otherwise idle while scalar's ACT
                # stream is a co-bottleneck. Slices are issued a full pair
                # (~8 quads) ahead, so gpsimd's slower triggers don't gate
                # anything. One 256KB slice per quad bounds the backlog.
                if q == 0:
                    xr_tiles[p] = xrp.tile([128, FLAT], F8, name="xr", tag="xr")
                xr_t = xr_tiles[p]
                sl = FLAT // 8
                (eng or nc.gpsimd).dma_start(
                    xr_t[:, q * sl : (q + 1) * sl], xr_hbm[p, :, q * sl : (q + 1) * sl]
                )

            # pair 0 is on the critical path at startup: fan its slices out
            # over two rings (consts ride sync) so the PE isn't data-starved.
            # Pair 1's first half rides scalar up front too -- pair 0's
            # quads run reduction-free and outpace the JIT prefetch.
            for q in range(8):
                load_pair_slice(0, q, (nc.scalar, nc.gpsimd)[q % 2])
            for q in range(4):
                load_pair_slice(1, q, nc.scalar)

            # pending_red = quad-lagged reduction work; pending_work =
            # (due_pair, due_qi, fn) items for the chunk-finals pipeline,
            # spread out so the fin-load lands quads before its tanhs ever
            # enter the ACT FIFO (head-of-line blocking the exps was worth
            # ~4.7us per chunk in the trace).
            pending_red = None
            pending_work = []
            fin = None
            stgv = None

            # conv slot map: (half, gl) -> (tile, offset, e-offset).
            # Three slots live in the 3-bank "main" tile; the 4th (h0,gl0)
            # lives in its own 1-bank tile that also receives the channel
            # sums + sv evac -- so the sv WAR chain never blocks the main
            # conv tile of the next quad (Tile WARs are tile-granular).
            def slot_info(half, gl, psm, psz):
                idx = {(0, 1): 0, (1, 0): 1, (1, 1): 2}.get((half, gl))
                if idx is None:
                    return psz, 0, 1536
                return psm, idx * 512, idx * 512

            def emit_conv(xrv, psm, psz, quad):
                # 12 matmuls, row-half interleaved for PE row-tiling
                # overlap; the psz slot is in the FIRST wave so its
                # Schraudolph evac can start a wave early.
                for wave in range(2):
                    for kw in range(3):
                        for half in range(2):
                            gl = half if wave == 0 else (1 - half)
                            rl, rh = 64 * half, 64 * half + 64
                            t0 = quad[gl][0]
                            pst, off, _ = slot_info(half, gl, psm, psz)
                            nc.tensor.matmul(
                                pst[:, off : off + 512],
                                w_sb[rl:rh, kw * 128 : (kw + 1) * 128],
                                xrv[rl:rh, t0 : t0 + 4, kw : kw + 128],
                                start=(kw == 0),
                                stop=(kw == 2),
                                tile_position=(64 * half, 0),
                                skip_group_check=True,
                            )

            def emit_reduce(psz, e, quad, qi, strip):
                # channel sums of quad q-1, written into the CURRENT quad's
                # psz bank (after its Schraudolph read): the sv WAR on the
                # psz pool then spans two generations and never stalls conv.
                # col-tiled [2, 512] matmuls into the psz bank
                for gl, (t0, cnt) in enumerate(quad):
                    for half in range(2):
                        _, _, eoff = slot_info(half, gl, None, None)
                        j = 32 * (2 * gl + half)
                        nc.tensor.matmul(
                            psz[j : j + 2, 0:512],
                            sel_sb[:, 0:2],
                            e[:, eoff : eoff + 512],
                            start=True,
                            stop=True,
                            tile_position=(0, j),
                            skip_group_check=True,
                        )
                # sparse evac of the sums on VectorE into the per-pair
                # strip, fused with the bit-pattern ln:
                # u = ln(s) ~= (bits(s)*2^-23 - B)*ln2
                # (copies some conv garbage on unused partitions; unread)
                ln2 = float(np.log(2.0))
                nc.vector.tensor_scalar(
                    strip[0:98, qi * 512 : (qi + 1) * 512],
                    psz[0:98, 0:512].bitcast(mybir.dt.int32),
                    ln2 / (1 << 23),
                    -(127.0 - 0.0430) * ln2,
                    mybir.AluOpType.mult,
                    mybir.AluOpType.add,
                )

            def emit_pair_stores(strip, wo_base, my_stgv, q_lo, q_hi, engs=None):
                # repack bounce 1/2: 4 bulk scatters per pair-half. The
                # d=0/d=1 rows ride in ONE transfer (partition dim 2 on
                # the SBUF side): each dma_start costs ~600ns on the issuing
                # engine's FIFO, so 4 triggers per half beat 8/64. Stage
                # rows are (d, gl, q, tt)-major so (q, tt) is contiguous
                # and the AP stays 3-dim after balancing. Emitted in two
                # q-halves so the ring load spreads out.
                for gl in range(2):
                    for half in range(2):
                        j = 32 * (2 * gl + half)
                        wo = wo_base + half * 128
                        sview = strip[j : j + 2, :].rearrange(
                            "o (q tt w) -> o q tt w", q=8, tt=4
                        )[:, q_lo:q_hi]
                        dview = my_stgv[:, gl, q_lo:q_hi, :, wo : wo + 128]
                        eng = engs[(2 * gl + half) % len(engs)] if engs else nc.sync
                        eng.dma_start(dview, sview)

            def sched_chunk_finals(my_fin, my_stg, my_pair):
                # finals pipeline for the 2-pair chunk on [126, 512]: fin
                # holds u = ln(s) (bit-pattern ln fused into the sv evac);
                # min' = -u/12 ; out = tanh(tanh(min')). Both tanhs come
                # from the ACT table -- the whole kernel stays in the
                # exp_and_others set (exp + tanh): no ACT_TABLE_LOAD
                # switches, no slow VectorE reciprocals. The three stages
                # are spread across the NEXT pair's quads so the tanhs
                # only enter the strict-FIFO ACT queue once their input is
                # already resident in SBUF.
                r = tmpp.tile([128, 512], F32, name="t", tag="t")
                o = tmpp.tile([128, 512], F32, name="t", tag="t")

                def load():
                    # repack bounce 2/2: one dense load per chunk
                    nc.sync.dma_start(my_fin[0:128, :], my_stg[:, :])

                def tanh1():
                    nc.scalar.activation(
                        r[0:128, :], my_fin[0:128, :],
                        mybir.ActivationFunctionType.Tanh,
                        scale=-1.0 / P_LSE,
                    )

                def tanh2():
                    nc.scalar.activation(
                        o[0:128, :], r[0:128, :],
                        mybir.ActivationFunctionType.Tanh,
                    )
                    nc.sync.dma_start(y_hbm[my_pair // 2], o[0:128, :])

                pending_work.append((my_pair + 1, 1, load))
                pending_work.append((my_pair + 1, 4, tanh1))
                pending_work.append((my_pair + 1, 5, tanh2))

            def sched_last_chunk_half(my_fin, my_stg, hw, due):
                # per-pair finals for the LAST chunk. Each pair has its OWN
                # [128, 256] stage+fin tiles (deps are tile-granular: with a
                # shared tile the pair-6 load would wait on pair-7's stores)
                # so the pair-6 half runs during pair 7's quads and only
                # pair 7's quarter rides the serial drain. hw picks the
                # output column half of the chunk's y row block.
                c0, c1 = hw * 256, hw * 256 + 256
                r = tmpp.tile([128, 256], F32, name="t", tag="t")
                o = tmpp.tile([128, 256], F32, name="t", tag="t")

                def load():
                    # gpsimd ring: it has no slice prefetches left by now
                    nc.gpsimd.dma_start(my_fin[0:128, :], my_stg[:, :])

                def tanh1():
                    nc.scalar.activation(
                        r[0:128, :], my_fin[0:128, :],
                        mybir.ActivationFunctionType.Tanh,
                        scale=-1.0 / P_LSE,
                    )

                def tanh2():
                    nc.scalar.activation(
                        o[0:128, :], r[0:128, :],
                        mybir.ActivationFunctionType.Tanh,
                    )
                    nc.sync.dma_start(y_hbm[NCHUNK - 1][:, c0:c1], o[0:128, :])

                pending_work.append((due[0], due[1], load))
                pending_work.append((due[0], due[1] + 2, tanh1))
                pending_work.append((due[0], due[1] + 3, tanh2))

            prev_fs = None
            for pair in range(PAIRS):
                xr = xr_tiles.pop(pair)
                # free dim as 64 double-rows of 256 (row r=2t at offset t*256)
                xrv = xr.rearrange("p (r q) -> p r q", q=2 * W)
                pl = pair % 2  # slot within the 2-pair finals chunk
                last_chunk = pair >= PAIRS - 2
                if pl == 0 or last_chunk:
                    width = 256 if last_chunk else 512
                    fin = finp.tile([128, width], F32, name="fin", tag="fin")
                    stg = stgp.tile([128, width], F32, name="stg", tag="stg")
                    stgv = stg.rearrange(
                        "(d gl q tt) w -> d gl q tt w", d=2, gl=2, q=8
                    )
                wo_base = 0 if last_chunk else 2 * pl * 128
                strip = svp.tile([128, 8 * 512], F32, name="strip", tag="strip")
                if pair == PAIRS - 1:
                    # pair-6 half of the last chunk: its stores land at
                    # (pair 7, qi 0); run load/tanhs during pair 7's quads
                    sched_last_chunk_half(prev_fs[0], prev_fs[1], 0, (pair, 1))

                for qi, quad in enumerate(QUADS):
                    psm = psmain.tile([128, 1536], F32, name="psm")
                    psz = psb0.tile([128, 512], F32, name="psz")
                    emit_conv(xrv, psm, psz, quad)
                    if pair + 1 < PAIRS and not (pair == 0 and qi < 4):
                        load_pair_slice(pair + 1, qi)
                    # e = exp(-12*(conv + b)), split across two engines:
                    # ScalarE Exp on the 3-slot main tile, VectorE
                    # Schraudolph bf16-exp on the psz slot (affine in f32,
                    # stored as int16 whose bits form bf16(e^z)).
                    e = ep.tile([128, 2048], BF16, name="e", tag="e")
                    nc.scalar.activation(
                        e[:, 0:1536],
                        psm[:, 0:1536],
                        mybir.ActivationFunctionType.Exp,
                        bias=b_sb[:, 0:1],
                        scale=-P_LSE / W_SCALE,
                    )
                    nc.vector.tensor_scalar(
                        e[:, 1536:2048].bitcast(mybir.dt.int16),
                        psz[:, 0:512],
                        -P_LSE / W_SCALE * 128.0 / float(np.log(2.0)),
                        b2_sb[:, 0:1],
                        mybir.AluOpType.mult,
                        mybir.AluOpType.add,
                    )
                    # quad-lagged reduction of the PREVIOUS quad, targeting
                    # THIS quad's psz tile (its schr read is done by then):
                    # exp(q-1) finished during this conv, so the sums never
                    # stall, and the sv WAR spans two psz generations.
                    if pending_red is not None:
                        p_e, p_quad, p_qi, p_strip, p_wo, p_stgv = pending_red
                        emit_reduce(psz, p_e, p_quad, p_qi, p_strip)
                        if p_qi == 3:
                            emit_pair_stores(p_strip, p_wo, p_stgv, 0, 4)
                        elif p_qi == 6 and pair == PAIRS - 1:
                            # flush the last pair's [4,7) strip quads early
                            # so the drain only waits on quad 7's quarter
                            emit_pair_stores(p_strip, p_wo, p_stgv, 4, 7)
                        elif p_qi == 7:
                            emit_pair_stores(p_strip, p_wo, p_stgv, 4, 8)
                    pending_red = (e, quad, qi, strip, wo_base, stgv)
                    # run any due finals-pipeline stages (load / tanh1 /
                    # tanh2+store), emitted at the tail of the quad so the
                    # quad's own exp sits ahead of them in the ACT FIFO.
                    for item in list(pending_work):
                        dp, dq, fn = item
                        if pair > dp or (pair == dp and qi >= dq):
                            fn()
                            pending_work.remove(item)

                if pl == 1:
                    if pair == PAIRS - 1:
                        sched_last_chunk_half(fin, stg, 1, (PAIRS, 0))
                    else:
                        sched_chunk_finals(fin, stg, pair)
                prev_fs = (fin, stg)


            # drain: last quad's reduction (fresh psz bank, zeroed so the
            # sv evac's garbage partitions are initialized) + last pair's
            # stores + the remaining finals stages in order
            if pending_red is not None:
                p_e, p_quad, p_qi, p_strip, p_wo, p_stgv = pending_red
                psz_last = psb0.tile([128, 512], F32, name="psz")
                nc.vector.memset(psz_last[:, :], 0.0)
                emit_reduce(psz_last, p_e, p_quad, p_qi, p_strip)
                # [4,7) was flushed during quad 7; only quad 7's quarter
                # left -- fan its 4 scatters over idle engine rings
                emit_pair_stores(
                    p_strip, p_wo, p_stgv, 7, 8,
                    engs=[nc.scalar, nc.sync, nc.gpsimd],
                )
            for _, _, fn in pending_work:
                fn()
            pending_work.clear()
    nc.finalize()
    return nc


_NC_CACHE = None


def _get_program():
    global _NC_CACHE
    if _NC_CACHE is None:
        _NC_CACHE = _build_program()
    return _NC_CACHE


def _host_prep(x, conv_weight, conv_bias):
    # x: [B, IC, H, W] f32
    # xrep[b, khe, ic, r, :] = x[b, ic, r+khe, :]  (zero past the end)
    xb = x.astype(ml_dtypes.float8_e4m3)
    xrep = np.zeros((B, 4, IC, H, W), dtype=ml_dtypes.float8_e4m3)
    for khe in range(4):
        xrep[:, khe, :, : H - khe, :] = xb[:, :, khe:, :]
    xrep = xrep.reshape(B, 4 * IC, FLAT)

    # weights: Wl[p=(khe*16+ic), kw, m=(delta*64+oc)] = w[oc, ic, khe-delta, kw]
    wl = np.zeros((64, 3, 128), dtype=np.float32)
    for khe in range(4):
        for dlt in range(2):
            kh = khe - dlt
            if 0 <= kh < KSZ:
                wl[khe * 16 : khe * 16 + 16, :, dlt * 64 : dlt * 64 + 64] = (
                    conv_weight[:, :, kh, :].transpose(1, 2, 0)
                )
    wts = np.concatenate([wl, wl], axis=0).reshape(128, 3 * 128) * W_SCALE
    wts = wts.astype(ml_dtypes.float8_e4m3)

    biasarr = np.tile(conv_bias.astype(np.float32), 2).reshape(128, 1)
    biasarr = biasarr * (-P_LSE)  # ACT bias applied after scale: exp(s*x + b)

    # Schraudolph bias for the VectorE exp path: bits(bf16 e^z) ~=
    # z*128/ln2 + 127*128 + c, z = -12*(psum + b). c centers the
    # mantissa-linear error and absorbs the f32->int16 truncation.
    bias2 = biasarr * (128.0 / np.log(2.0)) + (127.0 * 128.0 + 6.0)
    bias2 = bias2.astype(np.float32)

    # ones selector: sel[(d, oc), m] = (d == m)
    sel = np.zeros((128, 2), dtype=np.float32)
    sel[0:64, 0] = 1.0
    sel[64:128, 1] = 1.0
    sel = sel.astype(ml_dtypes.bfloat16)
    return xrep, wts, biasarr, bias2, sel


def _build_in_maps(x, conv_weight, conv_bias):
    xrep, wts, biasarr, bias2, sel = _host_prep(x, conv_weight, conv_bias)
    in_maps = []
    for c in range(N_CORES):
        xc = xrep[c * B_LOC : (c + 1) * B_LOC]  # [B_LOC, 64, FLAT]
        xc = np.ascontiguousarray(xc).reshape(PAIRS, 128, FLAT)
        in_maps.append(
            {"xrep": xc, "wts": wts, "bias": biasarr, "bias2": bias2, "sel": sel}
        )
    return in_maps


def kernel(x, conv_weight, conv_bias):
    x = np.asarray(x, dtype=np.float32)
    conv_weight = np.asarray(conv_weight, dtype=np.float32)
    conv_bias = np.asarray(conv_bias, dtype=np.float32)

    in_maps = _build_in_maps(x, conv_weight, conv_bias)
    nc = _get_program()
    res = run_bass_kernel_spmd(nc, in_maps, list(range(N_CORES)))
    # y: [NCHUNK, 128, 512]; rows (d 2, gl 2, q 8, tt 4) with
    # h' = 2*(8q + 4gl + tt) + d (t=63 dummy rows dropped);
    # cols (pair_loc 2, half 2, w 128); image b = chunk*4 + pl*2 + half
    ys = []
    for c in range(N_CORES):
        yc = res.results[c]["y"].reshape(NCHUNK, 2, 2, 8, 4, 2, 2, 128)
        # [chunk, d, gl, q, tt, pl, half, w] -> [chunk, pl, half, q, gl, tt, d, w]
        yc = yc.transpose(0, 5, 6, 3, 2, 4, 1, 7)
        ys.append(yc.reshape(B_LOC, 64, 2, 128))
    y = np.concatenate(ys, axis=0)  # [B, t, d, w]
    y = y.reshape(B, 128, 128)[:, :HO, :WO]  # (t,d) = h', drop dummies
    return np.ascontiguousarray(y).reshape(B, 1, HO, WO).astype(np.float32)

